# revision 14
# baseline (speedup 1.0000x reference)
"""Causal self-attention (B=2, S=4096, D=512, H=8) on 8 Trainium2 cores.

Sharding: core c handles batch b = c//4 and heads {2*(c%4), 2*(c%4)+1}.

Design (v3): k-major flash-style attention, restructured from v2 around the
PE tiling-mode drain: TRN2 drains the tensor engine whenever the array
tiling mode changes (64-row-tiled QK vs full-128 PV), so the inner loop is
PAIR-BATCHED into a [64-mode: QK x4] phase and a [128-mode: PV x4 + bg]
phase - one mode round-trip per TWO key blocks instead of two per block.

Other structural changes vs v2:
  - k/v projections (and V transposes) for ALL chunks run up front;
    attention chunks then run in REVERSE order (J=7..0) so the tail chunk
    is the small one and the heavy matmul streams run before DVFS throttle.
  - softmax division is deferred to the HOST: the kernel ships per-head
    unnormalized output projections (poT0/poT1, f32, DMA'd straight from
    PSUM - no engine evacuation) plus the denominator rows (den).  The
    per-head out-projections contract over 64 partitions, so the two heads
    run row-tiled CONCURRENTLY in the 64-mode phase.
  - the causal triangle mask is folded into the exp instruction itself:
    diagonal tiles use scalar_tensor_tensor((st*A) + TRI) where TRI is a
    host-precomputed [128,128] bias tile (0 / -1e9 + schraudolph bias), so
    the separate DVE tri-multiply disappears.
  - all scores carry a global offset -C (C=24) folded into the exp biases;
    it cancels in softmax and keeps the weights in fp8-friendly range.

The exp wall is split across ScalarE (native ACTIVATE exp) and VectorE
(one-instruction Schraudolph: int16(round(A*score + bias)) whose bit
pattern IS bf16 of 2^(log2e*score/8 + delta)).  Per-key exponent dither
delta decorrelates the interpolation error; V rows (and the den ones
column) are pre-scaled by 2^-delta so the dither cancels exactly in PV.
A greedy ns-balancer assigns each tile to whichever engine is less loaded.

PSUM budget (8 banks): st x4 (single-head score tiles, f32) | pv0 pv1 |
bg x2 (q-projection + per-head out-projection blocks, DMA'd from PSUM).
"""

import sys

sys.path.insert(0, "/opt/trn_rl_repo")

from contextlib import ExitStack

import ml_dtypes
import numpy as np

import concourse.bass as bass
import concourse.tile as tile
from concourse import bacc, bass_utils, mybir

B, S, D = 2, 4096, 512
H, HD = 8, 64
NCORES = 8
F32 = mybir.dt.float32
BF16 = mybir.dt.bfloat16
I16 = mybir.dt.int16
EXP = mybir.ActivationFunctionType.Exp
IDENT = mybir.ActivationFunctionType.Identity
MULT = mybir.AluOpType.mult
ADD = mybir.AluOpType.add
NPBF16 = ml_dtypes.bfloat16

CK = 512                      # query-chunk width
NCH = S // CK                 # 8
KBLK = 128                    # key block (partition dim)
KB_PER_CK = CK // KBLK        # 4
NEG = -1.0e30
LOG2E = 1.4426950408889634
A128 = 128 * LOG2E * 0.125    # DVE trick multiplier (raw-score units)
CSH = -0.045                  # Schraudolph shift
PHI = 0.6180339887498949
COFF = 24.0                   # global score offset (cancels in softmax)


class Balancer:
    """Greedy ns-accounting across ACT and DVE for balanceable ops."""

    def __init__(self, nc):
        self.nc = nc
        self.ns = {"act": 2700.0, "dve": 0.0}  # ACT pays the exp table load

    def _cost(self, eng, w):
        return (w + 352) / 1.2 if eng == "act" else (w + 150) / 0.96

    def charge(self, eng, w):
        self.ns[eng] += self._cost(eng, w)

    def pick(self, w):
        eng = "act" if self.ns["act"] + self._cost("act", w) <= \
            self.ns["dve"] + self._cost("dve", w) else "dve"
        self.charge(eng, w)
        return eng


def _emit(nc, tc, ctx, io):
    xT, wpack, fpack, poT = io

    bal = Balancer(nc)

    const = ctx.enter_context(tc.tile_pool(name="const", bufs=1))
    sb = ctx.enter_context(tc.tile_pool(name="sb", bufs=1))

    # ---- constants / weights ----
    cb = const.tile([128, 2240], BF16, tag="cbf16")
    cf = const.tile([128, 196], F32, tag="cf32")
    nc.sync.dma_start(cb[:, 0:512], wpack[:, 0:512])
    nc.sync.dma_start(cb[:, 512:2240], wpack[:, 512:2240])
    nc.sync.dma_start(cf[:], fpack[:])
    W_Q, W_K, W_V, W_O, ID2 = 0, 512, 1024, 1536, 2176
    BQKV, KB23, BACT, VSC, TRI16 = 0, 3, 35, 67, 68

    # ---- persistent SBUF ----
    kT = sb.tile([128, S], BF16, tag="kT")       # [2*64 hd, keys]
    # k-major V blocks padded to 128 cols (hd | ones@64 | junk pad); rows
    # 65+ of pv are junk
    v0 = sb.tile([128, 32 * 128], BF16, tag="v0")
    v1 = sb.tile([128, 32 * 128], BF16, tag="v1")

    xin = ctx.enter_context(tc.tile_pool(name="xin", bufs=3))
    qp = ctx.enter_context(tc.tile_pool(name="qp", bufs=NCH))
    vtp = ctx.enter_context(tc.tile_pool(name="vtp", bufs=2))
    etp = ctx.enter_context(tc.tile_pool(name="etp", bufs=8))
    otp = ctx.enter_context(tc.tile_pool(name="otp", bufs=2))
    pop = ctx.enter_context(tc.tile_pool(name="pop", bufs=2))
    rdp = ctx.enter_context(tc.tile_pool(name="rdp", bufs=2))

    ps_st = ctx.enter_context(tc.tile_pool(name="ps_st", bufs=1, space="PSUM"))
    ps_pv = ctx.enter_context(tc.tile_pool(name="ps_pv", bufs=1, space="PSUM"))
    ps_bg = ctx.enter_context(tc.tile_pool(name="ps_bg", bufs=2, space="PSUM"))

    # ones columns of v0/v1 (scaled 2^-delta)
    for vdst in (v0, v1):
        ones_col = vdst[:].rearrange("p (k c) -> p k c", c=128)[:, :, 64:65]
        nc.vector.tensor_copy(ones_col, cf[:, VSC:VSC + 1].to_broadcast((128, 32, 1)))

    # ---------------- background task machinery ----------------
    # bg64: tasks whose matmuls run in 64-row tiling mode (per-head outproj)
    # bg128: tasks whose matmuls run in full-128 mode (q projection)
    bg64, bg128 = [], []
    pace = {"c64": 0.0, "r64": 0.0, "c128": 0.0, "r128": 0.0}

    def drain64():
        pace["c64"] += pace["r64"]
        while pace["c64"] >= 1.0 and bg64:
            pace["c64"] -= 1.0
            bg64.pop(0)()

    def drain128():
        pace["c128"] += pace["r128"]
        while pace["c128"] >= 1.0 and bg128:
            pace["c128"] -= 1.0
            bg128.pop(0)()

    def copy_psum(dst_ap, src_ap, w, bias_col=None, scale=1.0):
        """psum->sbuf evacuation on the less-loaded of ACT/DVE."""
        eng = bal.pick(w)
        if eng == "act":
            if bias_col is not None:
                nc.scalar.activation(dst_ap, src_ap, IDENT, bias=bias_col,
                                     scale=scale)
            else:
                nc.scalar.copy(dst_ap, src_ap)
        else:
            if bias_col is not None:
                nc.vector.tensor_scalar(dst_ap, src_ap, scale, bias_col,
                                        MULT, ADD)
            else:
                nc.vector.tensor_copy(dst_ap, src_ap)

    # ---------------- projections ----------------
    q_tiles = {}

    def dma_x(J):
        xt = xin.tile([128, 4 * CK], BF16, tag="x")
        for ks in range(4):
            nc.sync.dma_start(
                xt[:, ks * CK:(ks + 1) * CK],
                xT[ks * 128:(ks + 1) * 128, J * CK:(J + 1) * CK])
        return xt

    def proj(xt, woff, bcol, dst_ap):
        """[128-mode] one projection: dst = W.T x^T + b for one chunk."""
        ps = ps_bg.tile([128, CK], F32, tag="bg")
        for ks in range(4):
            nc.tensor.matmul(
                ps[:], cb[:, woff + ks * 128:woff + (ks + 1) * 128],
                xt[:, ks * CK:(ks + 1) * CK],
                start=(ks == 0), stop=(ks == 3))
        copy_psum(dst_ap, ps[:], CK,
                  bias_col=cf[:, BQKV + bcol:BQKV + bcol + 1])

    def vtrans(J, vt, hh, vdst):
        """[64-mode] V transpose to k-major for one head."""
        ps = ps_bg.tile([128, CK], F32, tag="bg")
        tr = ps[:].bitcast(BF16)  # [128, 1024] bf16 view
        for i in range(4):
            nc.tensor.transpose(
                tr[:, i * 64:(i + 1) * 64],
                vt[hh * 64:(hh + 1) * 64, i * KBLK:(i + 1) * KBLK],
                cb[hh * 64:(hh + 1) * 64, ID2:ID2 + 64])
        dst = vdst[:, (J * 4) * 128:(J * 4 + 4) * 128]
        dst = dst.rearrange("p (k c) -> p k c", c=128)[:, :, 0:64]
        nc.vector.tensor_scalar_mul(
            dst, tr[:, 0:256].rearrange("p (k c) -> p k c", c=64),
            cf[:, VSC:VSC + 1])
        bal.charge("dve", 256)

    def emit_outproj(J):
        """Out-projection [128-mode] dt-blocks, evac'd + DMA'd."""
        oT = oT_tiles.pop(J)

        def mk(dt_):
            def f():
                ps = ps_bg.tile([128, CK], F32, tag="bg")
                nc.tensor.matmul(
                    ps[:], cb[:, W_O + dt_ * 128:W_O + (dt_ + 1) * 128],
                    oT[:], start=True, stop=True)
                po = pop.tile([128, CK], BF16, tag="po")
                copy_psum(po[:], ps[:], CK)
                nc.sync.dma_start(
                    poT[dt_ * 128:(dt_ + 1) * 128, J * CK:(J + 1) * CK],
                    po[:])
            return f
        for dt_ in range(4):
            bg128.append(mk(dt_))

    # ---------------- upfront q/k/v phase ----------------
    for J in range(NCH):
        xt = dma_x(J)
        csl = slice(J * CK, (J + 1) * CK)
        # [128-mode]
        proj(xt, W_K, 1, kT[:, csl])
        vt = vtp.tile([128, CK], BF16, tag="v")
        proj(xt, W_V, 2, vt[:])
        qt = qp.tile([128, CK], BF16, tag="q")
        q_tiles[J] = qt
        proj(xt, W_Q, 0, qt[:])
        # [64-mode]
        vtrans(J, vt, 0, v0)
        vtrans(J, vt, 1, v1)

    # ---------------- attention (reverse chunk order) ----------------
    oT_tiles = {}

    def emit_div(J, pv0t, pv1t):
        """den reciprocal + broadcast + oT divide (reads the pv psum)."""
        oT = otp.tile([128, CK], BF16, tag="oT")
        oT_tiles[J] = oT
        rdB = []
        for hh, pvt in ((0, pv0t), (1, pv1t)):
            den_t = rdp.tile([1, CK], F32, tag=f"den{hh}")
            nc.vector.tensor_copy(den_t[:], pvt[64:65, :])
            rd = rdp.tile([1, CK], F32, tag=f"rd{hh}")
            nc.vector.reciprocal_approx_fast(rd[:], den_t[:])
            bal.charge("dve", 2 * CK)
            rb = rdp.tile([64, CK], F32, tag=f"rdB{hh}")
            nc.gpsimd.partition_broadcast(rb[:], rd[:], channels=64)
            rdB.append(rb)
        for hh, pvt in ((0, pv0t), (1, pv1t)):
            hsl = slice(hh * 64, (hh + 1) * 64)
            nc.vector.tensor_mul(oT[hsl, :], pvt[0:64, :], rdB[hh][:])
            bal.charge("dve", CK)

    def emit_exp(kb, hh, st, et, col0, p):
        """exp of one [128, 512] score tile; diagonal tiles fold the causal
        triangle into the instruction via scalar_tensor_tensor."""
        w = CK - col0
        if p >= 0:
            # diagonal: triangle block via stt on DVE, remainder plain
            nc.vector.scalar_tensor_tensor(
                et[:, col0:col0 + KBLK].bitcast(I16), st[:, col0:col0 + KBLK],
                float(A128), cf[:, TRI16:TRI16 + KBLK], MULT, ADD)
            bal.charge("dve", KBLK)
            if col0 + KBLK < CK:
                rem = CK - col0 - KBLK
                if (eng := bal.pick(rem)) == "act":
                    nc.scalar.activation(
                        et[:, col0 + KBLK:], st[:, col0 + KBLK:], EXP,
                        bias=cf[:, BACT + kb:BACT + kb + 1], scale=0.125)
                else:
                    nc.vector.tensor_scalar(
                        et[:, col0 + KBLK:].bitcast(I16), st[:, col0 + KBLK:],
                        float(A128), cf[:, KB23 + kb:KB23 + kb + 1],
                        MULT, ADD)
        else:
            if (eng := bal.pick(w)) == "act":
                nc.scalar.activation(
                    et[:, col0:], st[:, col0:], EXP,
                    bias=cf[:, BACT + kb:BACT + kb + 1], scale=0.125)
            else:
                nc.vector.tensor_scalar(
                    et[:, col0:].bitcast(I16), st[:, col0:],
                    float(A128), cf[:, KB23 + kb:KB23 + kb + 1],
                    MULT, ADD)

    for J in range(NCH - 1, -1, -1):
        nkb = KB_PER_CK * (J + 1)
        npair = nkb // 2
        qt = q_tiles.pop(J)
        pv0t = ps_pv.tile([128, CK], F32, tag="pv0")
        pv1t = ps_pv.tile([128, CK], F32, tag="pv1")
        kb_ets = {}
        # pace bg tasks across this chunk's pair slots
        pace["r64"] = (len(bg64) + 0.5) / max(npair, 1)
        pace["r128"] = (len(bg128) * 4 + 0.5) / max(npair, 1)

        def emit_pv(kb):
            p = kb - KB_PER_CK * J
            col0 = KBLK * p if p >= 0 else 0
            for hh, vsb, pv in ((0, v0, pv0t), (1, v1, pv1t)):
                nc.tensor.matmul(
                    pv[:, col0:], vsb[:, kb * 128:(kb + 1) * 128],
                    kb_ets[kb][hh][:, col0:],
                    start=(kb == 0), stop=(kb == nkb - 1))
            del kb_ets[kb]

        for p in range(npair):
            # ---- 64-mode phase: QK for kbs (2p, 2p+1), bg64 tasks ----
            exps = []
            for kb in (2 * p, 2 * p + 1):
                pp = kb - KB_PER_CK * J
                col0 = KBLK * pp if pp >= 0 else 0
                ets = []
                for hh in range(2):
                    st = ps_st.tile([128, CK], F32, tag=f"st{hh}{kb % 2}")
                    hsl = slice(hh * 64, (hh + 1) * 64)
                    nc.tensor.matmul(
                        st[:, col0:], kT[hsl, kb * KBLK:(kb + 1) * KBLK],
                        qt[hsl, col0:], start=True, stop=True)
                    et = etp.tile([128, CK], BF16, tag="et")
                    ets.append(et)
                    exps.append((kb, hh, st, et, col0, pp))
                kb_ets[kb] = ets
            drain64()
            # ---- engine phase: exps (order = dependency order) ----
            for e in exps:
                emit_exp(*e)
            # ---- 128-mode phase: PV for previous pair, bg128 tasks ----
            if p >= 1:
                emit_pv(2 * p - 2)
                emit_pv(2 * p - 1)
            drain128()
        emit_pv(nkb - 2)
        emit_pv(nkb - 1)
        emit_div(J, pv0t, pv1t)
        emit_outproj(J)

    while bg64:
        bg64.pop(0)()
    while bg128:
        bg128.pop(0)()


_CACHED = None


def _build():
    global _CACHED
    if _CACHED is not None:
        return _CACHED
    nc = bacc.Bacc("TRN2", target_bir_lowering=False, debug=False,
                   enable_asserts=False, num_devices=NCORES)
    names = [
        ("xT", [D, S], BF16), ("wpack", [128, 2240], BF16),
        ("fpack", [128, 196], F32),
    ]
    aps = [nc.dram_tensor(n, sh, dt_, kind="ExternalInput").ap()
           for n, sh, dt_ in names]
    poT = nc.dram_tensor("poT", [D, S], BF16, kind="ExternalOutput").ap()
    with tile.TileContext(nc) as tc, ExitStack() as ctx:
        _emit(nc, tc, ctx, aps + [poT])
    nc.compile()
    _CACHED = nc
    return nc


def _host_inputs(x, attention_mask, Wq, bq, Wk, bk, Wv, bv, Wo, bo):
    f = np.float32
    x = np.asarray(x, f)
    mask = np.asarray(attention_mask)
    Wq, Wk, Wv, Wo = (np.asarray(w, f) for w in (Wq, Wk, Wv, Wo))
    bq, bk, bv = (np.asarray(b_, f) for b_ in (bq, bk, bv))
    id2 = np.tile(np.eye(64, dtype=NPBF16), (2, 1))
    delta = ((np.arange(128) * PHI) % 1.0).astype(f)          # per key%128
    vscale = (2.0 ** -delta)[:, None].astype(f)
    tri_keep = np.triu(np.ones((128, 128), bool))             # [k,q]: q >= k
    in_maps = []
    for c in range(NCORES):
        b = c // 4
        h0 = 2 * (c % 4)
        hsl = slice(64 * h0, 64 * h0 + 128)

        def pack_w(W):
            wt = W[hsl, :].T                        # [512, 128] = Wh^T
            return np.ascontiguousarray(
                wt.reshape(4, 128, 128).transpose(1, 0, 2)
                .reshape(128, 512).astype(NPBF16))

        wo_t = Wo[:, hsl].T.astype(NPBF16)           # [128, 512]
        mk = np.where(mask[b] != 0, f(0.0), f(NEG)).astype(f)  # [S]
        mk = mk.reshape(32, 128).T                   # [128 part, 32 kb]
        kb23 = (128.0 * (127.0 + CSH) + 128.0 * delta - A128 * COFF)[:, None] + \
            np.where(mk < 0, f(-1e9), f(0.0))
        biasact = (delta * np.log(2.0) - 0.125 * COFF)[:, None] + mk
        # diagonal-tile fused bias: schraudolph bias + causal -inf
        tri16 = np.where(tri_keep, kb23[:, 0:1], f(-1e9)).astype(f)  # [128,128]
        wpack = np.concatenate(
            [pack_w(Wq), pack_w(Wk), pack_w(Wv), wo_t,
             np.zeros((128, 128), NPBF16), id2], axis=1)
        fpack = np.concatenate(
            [np.stack([bq[hsl], bk[hsl], bv[hsl]], axis=1).astype(f),
             kb23.astype(f), biasact.astype(f), vscale, tri16], axis=1)

        in_maps.append({
            "xT": np.ascontiguousarray(x[b].T.astype(NPBF16)),
            "wpack": np.ascontiguousarray(wpack),
            "fpack": np.ascontiguousarray(fpack),
        })
    return in_maps


def _assemble(results, bo):
    out = np.zeros((B, S, D), np.float32)
    for c in range(NCORES):
        out[c // 4] += results[c]["poT"].astype(np.float32).T
    out += np.asarray(bo, np.float32)
    return out


def kernel(**inputs) -> np.ndarray:
    nc = _build()
    in_maps = _host_inputs(**inputs)
    last_err = None
    for attempt in range(3):
        try:
            res = bass_utils.run_bass_kernel_spmd(
                nc, in_maps, core_ids=list(range(NCORES)))
            out = _assemble(res.results, inputs["bo"])
        except Exception as e:  # transient NRT/axon device errors
            last_err = e
            continue
        if np.isfinite(out).all():
            return out
        last_err = RuntimeError("non-finite output")
    raise last_err


def run_traced(inputs, **kwargs):
    """test.py helper: run with NTFF tracing, return (out, BassKernelResults)."""
    nc = _build()
    in_maps = _host_inputs(**inputs)
    res = bass_utils.run_bass_kernel_spmd(
        nc, in_maps, core_ids=list(range(NCORES)), trace=True, **kwargs)
    return _assemble(res.results, inputs["bo"]), res
